# revision 18
# baseline (speedup 1.0000x reference)
"""Bidirectional toroidal lattice message passing on 8 Trainium2 cores.

The [N,N] adjacencies produced by this model are toroidal 3-neighbor shift
operators (3 constant-value generalized diagonals, zero elsewhere). We verify
that structure on the host, extract the per-shift constants, and run the
10-step propagation fully on-chip:
  - partition dim = theta (128), free = (dir, batch, phi) = [128, 2*2*64]
  - theta-shift: 128x128 scaled-permutation matmul on TensorE (PSUM accum)
  - phi-shift:   offset copies on GPSIMD
  - per-step state update + weighted accumulation on VectorE
Batch is sharded 2-per-core across 8 cores; no collectives needed.
"""

import numpy as np

NT, NP, S = 128, 64, 10
N = NT * NP
B = 16
NCORES = 8
BPC = B // NCORES  # batches per core

_FWD = [(1, 0), (0, 1), (1, 1)]
_REV = [(-1, 0), (0, -1), (-1, -1)]


def _diag_vals(adj, shifts):
    idx = np.arange(N)
    ti, pi = idx // NP, idx % NP
    return [adj[idx, ((ti + dt) % NT) * NP + (pi + dp) % NP] for dt, dp in shifts]


def _softmax(x):
    e = np.exp(x - x.max())
    return (e / e.sum()).astype(np.float32)


def _structure_ok(adj, vals):
    # every extracted diagonal constant, and no off-diagonal mass
    for v in vals:
        if np.ptp(v) > 1e-6 * max(1.0, abs(float(v.mean()))):
            return False
    total = adj.sum(dtype=np.float64)
    diag = sum(v.sum(dtype=np.float64) for v in vals)
    return abs(total - diag) < 1e-3


def _reference_fallback(entry, fwd_adj, rev_adj, fwd_sw, fwd_decay, rev_sw,
                        rev_decay, iw, angles):
    # generic dense path (host); only used if the adjacency is not the
    # expected toroidal shift structure.
    def prop(adj, decay, sw):
        d = float(np.clip(decay, 0.5, 0.99))
        af = 0.5 + 0.5 * np.cos(np.abs(angles).mean(axis=1))
        x = entry.astype(np.float32)
        w = _softmax(np.asarray(sw, np.float32))
        acc = np.zeros_like(x)
        for s in range(S):
            p = (x @ adj) * af[None, :]
            x = ((0.3 * x + 0.7 * p) * d).astype(np.float32)
            acc += w[s] * x
        return acc
    f = prop(fwd_adj, fwd_decay, fwd_sw)
    r = prop(rev_adj, rev_decay, rev_sw)
    inter = f * r
    sig = 1.0 / (1.0 + np.exp(-float(iw)))
    return (f + r + np.float32(sig) * inter).astype(np.float32), inter.astype(np.float32)


def _build_program(c1, w, sig_w, g_scale, pm_keys):
    """Build the SPMD Bass program (identical on all cores).

    c1:      (c1_f, c1_r) floats
    w:       (w_f[10], w_r[10]) softmaxed step weights
    g_scale: (gs_f, gs_r) = 0.35 * decay for each dir
    pm_keys: names of the 5-6 stationary-matrix inputs:
             {'fa','fb','fi','ra','rb','ri'} possibly deduped to same name
    """
    import concourse.bacc as bacc
    import concourse.mybir as mybir
    from concourse.tile import TileContext

    fp32 = mybir.dt.float32
    AF = mybir.ActivationFunctionType
    OP = mybir.AluOpType

    nc = bacc.Bacc(None, target_bir_lowering=False)

    # one packed constant input -> one DMA -> one semaphore for all consumers
    pm_names = sorted(set(pm_keys.values()))
    n_pm = len(pm_names)
    CW = n_pm * NT + NP * 3 + 1 + 2 * BPC * NP
    consts_d = nc.dram_tensor("consts", [NT, CW], fp32, kind="ExternalInput")
    out_c = nc.dram_tensor("out_combined", [BPC, N], fp32, kind="ExternalOutput")
    out_i = nc.dram_tensor("out_interaction", [BPC, N], fp32, kind="ExternalOutput")

    (c1f, c1r), (wf, wr), (gsf, gsr) = c1, w, g_scale
    same_c1 = abs(c1f - c1r) < 1e-12
    same_w = all(abs(a - b) < 1e-12 for a, b in zip(wf, wr))
    same_g = abs(gsf - gsr) < 1e-12

    with TileContext(nc) as tc:
        with (
            tc.tile_pool(name="const", bufs=1) as cpool,
            tc.tile_pool(name="state", bufs=2) as spool,
            tc.tile_pool(name="work", bufs=2) as wpool,
            tc.tile_pool(name="accp", bufs=2) as apool,
            tc.tile_pool(name="psum", bufs=2, space="PSUM") as ppool,
        ):
            # ---- load all constants with one DMA ----
            consts = cpool.tile([NT, CW], fp32, tag="consts")
            nc.sync.dma_start(consts[:], consts_d[:])
            pm = {nm: consts[:, i * NT:(i + 1) * NT]
                  for i, nm in enumerate(pm_names)}
            ang_off = n_pm * NT
            pi2_off = ang_off + NP * 3
            x0_off = pi2_off + 1
            ang = consts[:, ang_off:pi2_off].rearrange("t (p k) -> t p k", k=3)

            # ---- angle factor -> per-cell gains g_e [128, 2, BPC, 64] ----
            ab = cpool.tile([NT, NP, 3], fp32, tag="ab")
            nc.scalar.activation(ab[:], ang, AF.Abs)
            s1 = cpool.tile([NT, NP], fp32, tag="s1")
            nc.vector.tensor_add(s1[:], ab[:, :, 0], ab[:, :, 1])
            nc.vector.tensor_add(s1[:], s1[:], ab[:, :, 2])
            af0 = cpool.tile([NT, NP], fp32, tag="af0")
            # cos(mean) = sin(mean + pi/2), mean = s1/3 ; pi/2 lives in consts
            nc.scalar.activation(af0[:], s1[:], AF.Sin,
                                 bias=consts[:, pi2_off:pi2_off + 1],
                                 scale=1.0 / 3.0)
            # g = 0.35*d*(1 + af0)
            g64f = cpool.tile([NT, NP], fp32, tag="g64f")
            nc.scalar.activation(g64f[:], af0[:], AF.Copy, bias=gsf, scale=gsf)
            if same_g:
                g64r = g64f
            else:
                g64r = cpool.tile([NT, NP], fp32, tag="g64r")
                nc.scalar.activation(g64r[:], af0[:], AF.Copy, bias=gsr, scale=gsr)
            g_e = cpool.tile([NT, 2, BPC, NP], fp32, tag="g_e")
            for d, g64 in ((0, g64f), (1, g64r)):
                for b in range(BPC):
                    nc.gpsimd.tensor_copy(g_e[:, d, b, :], g64[:])

            # ---- initial state: read directly from the packed const tile ----
            x = consts[:, x0_off:x0_off + 2 * BPC * NP].rearrange(
                "t (d b p) -> t d b p", d=2, b=BPC)

            acc = None
            for s in range(S):
                # phi-shifted copy of x (fwd: p-1 -> p, rev: p+1 -> p)
                xp = wpool.tile([NT, 2, BPC, NP], fp32, tag="xp")
                nc.gpsimd.tensor_copy(xp[:, 0, :, 1:NP], x[:, 0, :, 0:NP - 1])
                nc.gpsimd.tensor_copy(xp[:, 0, :, 0:1], x[:, 0, :, NP - 1:NP])
                nc.gpsimd.tensor_copy(xp[:, 1, :, 0:NP - 1], x[:, 1, :, 1:NP])
                nc.gpsimd.tensor_copy(xp[:, 1, :, NP - 1:NP], x[:, 1, :, 0:1])

                # prop = v10*roll_theta(x) + v11*roll_theta(xp) + v01*xp
                ps = ppool.tile([NT, 2, BPC, NP], fp32, tag="ps")
                for d, (ka, kb, ki) in ((0, ("fa", "fb", "fi")),
                                        (1, ("ra", "rb", "ri"))):
                    nc.tensor.matmul(ps[:, d], pm[pm_keys[ka]], x[:, d],
                                     start=True, stop=False)
                    nc.tensor.matmul(ps[:, d], pm[pm_keys[kb]], xp[:, d],
                                     start=False, stop=False)
                    nc.tensor.matmul(ps[:, d], pm[pm_keys[ki]], xp[:, d],
                                     start=False, stop=True)

                # tmp = prop * g_e ; x' = c1*x + tmp ; acc += w_s * x'
                tmp = wpool.tile([NT, 2, BPC, NP], fp32, tag="tmp")
                nc.vector.tensor_mul(tmp[:], ps[:], g_e[:])
                xn = spool.tile([NT, 2, BPC, NP], fp32, tag="x")
                if same_c1:
                    nc.vector.scalar_tensor_tensor(
                        xn[:], x[:], c1f, tmp[:], op0=OP.mult, op1=OP.add)
                else:
                    for d, c in ((0, c1f), (1, c1r)):
                        nc.vector.scalar_tensor_tensor(
                            xn[:, d], x[:, d], c, tmp[:, d],
                            op0=OP.mult, op1=OP.add)
                x = xn

                an = apool.tile([NT, 2, BPC, NP], fp32, tag="acc")
                if acc is None:
                    if same_w:
                        nc.vector.tensor_scalar_mul(an[:], x[:], wf[0])
                    else:
                        nc.vector.tensor_scalar_mul(an[:, 0], x[:, 0], wf[0])
                        nc.vector.tensor_scalar_mul(an[:, 1], x[:, 1], wr[0])
                else:
                    if same_w:
                        nc.vector.scalar_tensor_tensor(
                            an[:], x[:], wf[s], acc[:], op0=OP.mult, op1=OP.add)
                    else:
                        for d, ws in ((0, wf[s]), (1, wr[s])):
                            nc.vector.scalar_tensor_tensor(
                                an[:, d], x[:, d], ws, acc[:, d],
                                op0=OP.mult, op1=OP.add)
                acc = an

            # ---- combine: inter = f*r, comb = f + r + sig*inter ----
            inter = wpool.tile([NT, BPC, NP], fp32, tag="inter")
            nc.vector.tensor_mul(inter[:], acc[:, 0], acc[:, 1])
            fr = wpool.tile([NT, BPC, NP], fp32, tag="fr")
            nc.vector.tensor_add(fr[:], acc[:, 0], acc[:, 1])
            comb = wpool.tile([NT, BPC, NP], fp32, tag="comb")
            nc.vector.scalar_tensor_tensor(
                comb[:], inter[:], sig_w, fr[:], op0=OP.mult, op1=OP.add)

            oc = out_c[:].rearrange("b (t p) -> b t p", t=NT)
            oi = out_i[:].rearrange("b (t p) -> b t p", t=NT)
            for b in range(BPC):
                nc.sync.dma_start(oc[b], comb[:, b, :])
                nc.sync.dma_start(oi[b], inter[:, b, :])

    nc.finalize()
    return nc


def _host_prep(inputs):
    entry = np.ascontiguousarray(np.asarray(inputs["entry_probs"], np.float32))
    fwd_adj = np.asarray(inputs["forward_adj"], np.float32)
    rev_adj = np.asarray(inputs["reverse_adj"], np.float32)
    angles = np.ascontiguousarray(np.asarray(inputs["bounce_angles"], np.float32))

    vf = _diag_vals(fwd_adj, _FWD)
    vr = _diag_vals(rev_adj, _REV)
    ok = _structure_ok(fwd_adj, vf) and _structure_ok(rev_adj, vr)

    df = float(np.clip(float(np.asarray(inputs["forward_decay"])), 0.5, 0.99))
    dr = float(np.clip(float(np.asarray(inputs["reverse_decay"])), 0.5, 0.99))
    wf = _softmax(np.asarray(inputs["forward_step_weights"], np.float32))
    wr = _softmax(np.asarray(inputs["reverse_step_weights"], np.float32))
    sig = float(1.0 / (1.0 + np.exp(-float(np.asarray(inputs["interaction_weight"])))))

    vbf = [float(v.mean()) for v in vf]
    vbr = [float(v.mean()) for v in vr]

    # stationary matrices: k = source partition, i = dest partition
    k = np.arange(NT)
    p_fwd = np.zeros((NT, NT), np.float32)   # out[t] = in[t-1]
    p_fwd[(k - 1) % NT, k] = 1.0
    p_rev = np.zeros((NT, NT), np.float32)   # out[t] = in[t+1]
    p_rev[(k + 1) % NT, k] = 1.0
    eye = np.eye(NT, dtype=np.float32)

    mats = {
        "fa": vbf[0] * p_fwd, "fb": vbf[2] * p_fwd, "fi": vbf[1] * eye,
        "ra": vbr[0] * p_rev, "rb": vbr[2] * p_rev, "ri": vbr[1] * eye,
    }
    # dedupe identical matrices so the PE can keep weights loaded
    pm_keys, pm_data, seen = {}, {}, {}
    for kk, m in mats.items():
        h = m.tobytes()
        if h in seen:
            pm_keys[kk] = seen[h]
        else:
            nm = f"pm_{kk}"
            seen[h] = nm
            pm_keys[kk] = nm
            pm_data[nm] = m

    # packed consts per core: [pm matrices (sorted name order) | angles | x0]
    # x0 = [128, dir(2), b(2), 64], entry duplicated across dir
    ang = angles.reshape(NT, NP * 3)
    pm_sorted = [pm_data[nm] for nm in sorted(pm_data)]
    pi2_col = np.full((NT, 1), np.pi / 2, np.float32)
    consts_common = np.concatenate(pm_sorted + [ang, pi2_col], axis=1)
    consts_list = []
    for c in range(NCORES):
        e = entry[c * BPC:(c + 1) * BPC].reshape(BPC, NT, NP).transpose(1, 0, 2)
        x0 = np.broadcast_to(e[:, None], (NT, 2, BPC, NP))
        x0 = np.ascontiguousarray(x0).reshape(NT, 2 * BPC * NP)
        consts_list.append(
            np.ascontiguousarray(np.concatenate([consts_common, x0], axis=1)))

    meta = dict(
        ok=ok,
        c1=(0.3 * df, 0.3 * dr),
        w=(list(map(float, wf)), list(map(float, wr))),
        g_scale=(0.35 * df, 0.35 * dr),
        sig=sig, pm_keys=pm_keys, n_pm=len(pm_data), consts_list=consts_list,
    )
    return meta


_PROGRAM_CACHE = {}
LAST_RESULT = None


def kernel(**inputs):
    meta = _host_prep(inputs)
    if not meta["ok"]:
        return _reference_fallback(
            np.asarray(inputs["entry_probs"], np.float32),
            np.asarray(inputs["forward_adj"], np.float32),
            np.asarray(inputs["reverse_adj"], np.float32),
            inputs["forward_step_weights"], inputs["forward_decay"],
            inputs["reverse_step_weights"], inputs["reverse_decay"],
            inputs["interaction_weight"], np.asarray(inputs["bounce_angles"], np.float32))

    from concourse import bass_utils

    key = (tuple(meta["c1"]), tuple(meta["w"][0]), tuple(meta["w"][1]),
           meta["sig"], tuple(sorted(meta["pm_keys"].items())),
           tuple(meta["g_scale"]))
    if key not in _PROGRAM_CACHE:
        _PROGRAM_CACHE[key] = _build_program(
            meta["c1"], meta["w"], meta["sig"], meta["g_scale"], meta["pm_keys"])
    nc = _PROGRAM_CACHE[key]

    in_maps = [{"consts": meta["consts_list"][c]} for c in range(NCORES)]

    res = bass_utils.run_bass_kernel_spmd(nc, in_maps, core_ids=list(range(NCORES)))
    global LAST_RESULT
    LAST_RESULT = res

    combined = np.concatenate([r["out_combined"] for r in res.results], axis=0)
    interaction = np.concatenate([r["out_interaction"] for r in res.results], axis=0)
    return combined, interaction


# revision 22
# speedup vs baseline: 1.2393x; 1.2393x over previous
"""Bidirectional toroidal lattice message passing on 8 Trainium2 cores.

The [N,N] adjacencies produced by this model are toroidal 3-neighbor shift
operators (3 constant-value generalized diagonals, zero elsewhere). We verify
that structure on the host, extract the per-shift constants, and run the
10-step propagation fully on-chip:

  - partition dim = theta (128); free dims = (dir 2, batch 2, phi 64+halo)
  - theta-shift terms via two 128x128 stationary matmuls per direction
    accumulated in PSUM:  P_d = v10*Ptheta_d  and  M_d = v11*Ptheta_d + v01*I
    applied to x and to the phi-shifted view of x
  - phi-shift is free: x tiles carry 2 halo columns kept up to date with two
    tiny copies per step, so the shifted operand is just a strided view
  - per-step DVE: tmp = psum * g_e ; x' = c1*x + tmp   (g_e = 0.7*decay*af
    host-prepacked); step accumulation acc += w_s * x' runs on GPSIMD

Batch is sharded 2-per-core across 8 cores; no collectives needed.
"""

import numpy as np

NT, NP, S = 128, 64, 10
N = NT * NP
B = 16
NCORES = 8
BPC = B // NCORES  # batches per core
NH = NP + 2        # phi width incl. wrap halos: [wrap_pre | 0..63 | wrap_post]

_FWD = [(1, 0), (0, 1), (1, 1)]
_REV = [(-1, 0), (0, -1), (-1, -1)]


def _diag_vals(adj, shifts):
    idx = np.arange(N)
    ti, pi = idx // NP, idx % NP
    return [adj[idx, ((ti + dt) % NT) * NP + (pi + dp) % NP] for dt, dp in shifts]


def _softmax(x):
    e = np.exp(x - x.max())
    return (e / e.sum()).astype(np.float32)


def _structure_ok(adj, vals):
    for v in vals:
        if np.ptp(v) > 1e-6 * max(1.0, abs(float(v.mean()))):
            return False
    total = adj.sum(dtype=np.float64)
    diag = sum(v.sum(dtype=np.float64) for v in vals)
    return abs(total - diag) < 1e-3


def _reference_fallback(entry, fwd_adj, rev_adj, fwd_sw, fwd_decay, rev_sw,
                        rev_decay, iw, angles):
    # generic dense path (host); only used if the adjacency is not the
    # expected toroidal shift structure.
    def prop(adj, decay, sw):
        d = float(np.clip(decay, 0.5, 0.99))
        af = 0.5 + 0.5 * np.cos(np.abs(angles).mean(axis=1))
        x = entry.astype(np.float32)
        w = _softmax(np.asarray(sw, np.float32))
        acc = np.zeros_like(x)
        for s in range(S):
            p = (x @ adj) * af[None, :]
            x = ((0.3 * x + 0.7 * p) * d).astype(np.float32)
            acc += w[s] * x
        return acc
    f = prop(fwd_adj, fwd_decay, fwd_sw)
    r = prop(rev_adj, rev_decay, rev_sw)
    inter = f * r
    sig = 1.0 / (1.0 + np.exp(-float(iw)))
    return (f + r + np.float32(sig) * inter).astype(np.float32), inter.astype(np.float32)


def _build_program(c1, w, sig_w, use_fp32r=True):
    """SPMD Bass program (identical on all cores).

    consts layout (free dim):
      [P_f | M_f | P_r | M_r  (128 each) | g_e 2*BPC*64 | x0 2*BPC*NH]
    """
    import concourse.bacc as bacc
    import concourse.mybir as mybir
    from concourse.tile import TileContext

    fp32 = mybir.dt.float32
    mm_dt = mybir.dt.float32r if use_fp32r else fp32
    OP = mybir.AluOpType

    nc = bacc.Bacc(None, target_bir_lowering=False)

    GE = 2 * BPC * NP
    XW = 2 * BPC * NH
    CW = 4 * NT + GE + XW
    consts_d = nc.dram_tensor("consts", [NT, CW], fp32, kind="ExternalInput")
    out_d = nc.dram_tensor("out_all", [2, BPC, N], fp32, kind="ExternalOutput")

    (c1f, c1r), (wf, wr) = c1, w
    same_c1 = abs(c1f - c1r) < 1e-12
    same_w = all(abs(a - b) < 1e-12 for a, b in zip(wf, wr))

    g_off = 4 * NT
    x0_off = g_off + GE

    with TileContext(nc) as tc:
        with (
            tc.tile_pool(name="const", bufs=1) as cpool,
            tc.tile_pool(name="state", bufs=2) as spool,
            tc.tile_pool(name="work", bufs=2) as wpool,
            tc.tile_pool(name="accp", bufs=2) as apool,
            tc.tile_pool(name="psum", bufs=2, space="PSUM") as ppool,
        ):
            consts = cpool.tile([NT, CW], fp32, tag="consts")
            nc.sync.dma_start(consts[:], consts_d[:])
            pm = {k: consts[:, i * NT:(i + 1) * NT].bitcast(mm_dt)
                  for i, k in enumerate(("Pf", "Mf", "Pr", "Mr"))}
            g_e = consts[:, g_off:g_off + GE].rearrange(
                "t (d b p) -> t d b p", d=2, b=BPC)

            # initial state: host-packed with halos, read in place
            x = consts[:, x0_off:x0_off + XW].rearrange(
                "t (d b p) -> t d b p", d=2, b=BPC)

            acc = None
            for s in range(S):
                # prop_d = P_d @ x_d + M_d @ xphi_d   (PSUM accumulation)
                # xphi view: fwd reads cols 0..63 (p-1), rev reads 2..65 (p+1)
                ps = ppool.tile([NT, 2, BPC, NP], fp32, tag="ps")
                xmm = x.bitcast(mm_dt)
                nc.tensor.matmul(ps[:, 0], pm["Pf"], xmm[:, 0, :, 1:NP + 1],
                                 start=True, stop=False)
                nc.tensor.matmul(ps[:, 0], pm["Mf"], xmm[:, 0, :, 0:NP],
                                 start=False, stop=True)
                nc.tensor.matmul(ps[:, 1], pm["Pr"], xmm[:, 1, :, 1:NP + 1],
                                 start=True, stop=False)
                nc.tensor.matmul(ps[:, 1], pm["Mr"], xmm[:, 1, :, 2:NP + 2],
                                 start=False, stop=True)

                # tmp = prop * g_e ; x'_center = c1*x + tmp
                tmp = wpool.tile([NT, 2, BPC, NP], fp32, tag="tmp")
                nc.vector.tensor_mul(tmp[:], ps[:], g_e)
                xn = spool.tile([NT, 2, BPC, NH], fp32, tag="x")
                xc = xn[:, :, :, 1:NP + 1]
                if same_c1:
                    nc.vector.scalar_tensor_tensor(
                        xc, x[:, :, :, 1:NP + 1], c1f, tmp[:],
                        op0=OP.mult, op1=OP.add)
                else:
                    for d, c in ((0, c1f), (1, c1r)):
                        nc.vector.scalar_tensor_tensor(
                            xn[:, d, :, 1:NP + 1], x[:, d, :, 1:NP + 1], c,
                            tmp[:, d], op0=OP.mult, op1=OP.add)
                # refresh wrap halos (tiny)
                nc.gpsimd.tensor_copy(xn[:, :, :, 0:1], xn[:, :, :, NP:NP + 1])
                nc.gpsimd.tensor_copy(xn[:, :, :, NP + 1:NP + 2], xn[:, :, :, 1:2])
                x = xn[:]

                # acc += w_s * x'
                an = apool.tile([NT, 2, BPC, NP], fp32, tag="acc")
                if acc is None:
                    if same_w:
                        nc.vector.tensor_scalar_mul(an[:], xc, wf[0])
                    else:
                        nc.vector.tensor_scalar_mul(an[:, 0], xn[:, 0, :, 1:NP + 1], wf[0])
                        nc.vector.tensor_scalar_mul(an[:, 1], xn[:, 1, :, 1:NP + 1], wr[0])
                else:
                    if same_w:
                        nc.vector.scalar_tensor_tensor(
                            an[:], xc, wf[s], acc[:], op0=OP.mult, op1=OP.add)
                    else:
                        for d, ws in ((0, wf[s]), (1, wr[s])):
                            nc.vector.scalar_tensor_tensor(
                                an[:, d], xn[:, d, :, 1:NP + 1], ws, acc[:, d],
                                op0=OP.mult, op1=OP.add)
                acc = an

            # combine: inter = f*r ; comb = f + r + sig*inter ; one output DMA
            out2 = wpool.tile([NT, 2, BPC, NP], fp32, tag="out2")
            inter = out2[:, 1]
            nc.vector.tensor_mul(inter, acc[:, 0], acc[:, 1])
            fr = wpool.tile([NT, BPC, NP], fp32, tag="fr")
            nc.vector.tensor_add(fr[:], acc[:, 0], acc[:, 1])
            nc.vector.scalar_tensor_tensor(
                out2[:, 0], inter, sig_w, fr[:], op0=OP.mult, op1=OP.add)
            nc.sync.dma_start(
                out_d[:].rearrange("o b (t p) -> t o b p", t=NT), out2[:])

    nc.finalize()
    return nc


def _host_prep(inputs):
    entry = np.ascontiguousarray(np.asarray(inputs["entry_probs"], np.float32))
    fwd_adj = np.asarray(inputs["forward_adj"], np.float32)
    rev_adj = np.asarray(inputs["reverse_adj"], np.float32)
    angles = np.asarray(inputs["bounce_angles"], np.float32)

    vf = _diag_vals(fwd_adj, _FWD)
    vr = _diag_vals(rev_adj, _REV)
    ok = _structure_ok(fwd_adj, vf) and _structure_ok(rev_adj, vr)

    df = float(np.clip(float(np.asarray(inputs["forward_decay"])), 0.5, 0.99))
    dr = float(np.clip(float(np.asarray(inputs["reverse_decay"])), 0.5, 0.99))
    wf = _softmax(np.asarray(inputs["forward_step_weights"], np.float32))
    wr = _softmax(np.asarray(inputs["reverse_step_weights"], np.float32))
    sig = float(1.0 / (1.0 + np.exp(-float(np.asarray(inputs["interaction_weight"])))))

    vbf = [float(v.mean()) for v in vf]   # [v10, v01, v11]
    vbr = [float(v.mean()) for v in vr]

    k = np.arange(NT)
    p_fwd = np.zeros((NT, NT), np.float32)   # out[t] = in[t-1]
    p_fwd[(k - 1) % NT, k] = 1.0
    p_rev = np.zeros((NT, NT), np.float32)   # out[t] = in[t+1]
    p_rev[(k + 1) % NT, k] = 1.0
    eye = np.eye(NT, dtype=np.float32)

    mats = [vbf[0] * p_fwd, vbf[2] * p_fwd + vbf[1] * eye,
            vbr[0] * p_rev, vbr[2] * p_rev + vbr[1] * eye]

    # per-cell gain g = 0.7 * decay * (0.5 + 0.5*cos(mean|angles|)),
    # expanded to [128, dir, b, 64]
    af = (0.5 + 0.5 * np.cos(np.abs(angles).mean(axis=1))).astype(np.float32)
    af2 = af.reshape(NT, NP)
    g_e = np.empty((NT, 2, BPC, NP), np.float32)
    g_e[:, 0] = (0.7 * df) * af2[:, None, :]
    g_e[:, 1] = (0.7 * dr) * af2[:, None, :]

    consts_common = np.concatenate(
        [m for m in mats] + [g_e.reshape(NT, -1)], axis=1)

    consts_list = []
    for c in range(NCORES):
        e = entry[c * BPC:(c + 1) * BPC].reshape(BPC, NT, NP).transpose(1, 0, 2)
        x0 = np.empty((NT, 2, BPC, NH), np.float32)
        x0[:, :, :, 1:NP + 1] = e[:, None]
        x0[:, :, :, 0] = e[:, None, :, NP - 1]
        x0[:, :, :, NP + 1] = e[:, None, :, 0]
        consts_list.append(np.ascontiguousarray(
            np.concatenate([consts_common, x0.reshape(NT, -1)], axis=1)))

    meta = dict(
        ok=ok,
        c1=(0.3 * df, 0.3 * dr),
        w=(list(map(float, wf)), list(map(float, wr))),
        sig=sig, consts_list=consts_list,
    )
    return meta


_PROGRAM_CACHE = {}
LAST_RESULT = None
USE_FP32R = False


def kernel(**inputs):
    meta = _host_prep(inputs)
    if not meta["ok"]:
        return _reference_fallback(
            np.asarray(inputs["entry_probs"], np.float32),
            np.asarray(inputs["forward_adj"], np.float32),
            np.asarray(inputs["reverse_adj"], np.float32),
            inputs["forward_step_weights"], inputs["forward_decay"],
            inputs["reverse_step_weights"], inputs["reverse_decay"],
            inputs["interaction_weight"], np.asarray(inputs["bounce_angles"], np.float32))

    from concourse import bass_utils

    key = (tuple(meta["c1"]), tuple(meta["w"][0]), tuple(meta["w"][1]),
           meta["sig"], USE_FP32R)
    if key not in _PROGRAM_CACHE:
        _PROGRAM_CACHE[key] = _build_program(
            meta["c1"], meta["w"], meta["sig"], use_fp32r=USE_FP32R)
    nc = _PROGRAM_CACHE[key]

    in_maps = [{"consts": meta["consts_list"][c]} for c in range(NCORES)]
    res = bass_utils.run_bass_kernel_spmd(nc, in_maps, core_ids=list(range(NCORES)))
    global LAST_RESULT
    LAST_RESULT = res

    combined = np.concatenate([r["out_all"][0] for r in res.results], axis=0)
    interaction = np.concatenate([r["out_all"][1] for r in res.results], axis=0)
    return combined, interaction


# revision 25
# speedup vs baseline: 1.6638x; 1.3425x over previous
"""Bidirectional toroidal lattice message passing on 8 Trainium2 cores.

The [N,N] adjacencies produced by this model are toroidal 3-neighbor shift
operators (3 constant-value generalized diagonals, zero elsewhere). We verify
that structure on the host, extract the per-shift constants, and run the
10-step propagation fully on-chip:

  - partition dim = theta (128); free dims = (dir 2, batch 2, phi 64+halo)
  - theta-shift terms via two 128x128 stationary matmuls per direction
    accumulated in PSUM:  P_d = v10*Ptheta_d  and  M_d = v11*Ptheta_d + v01*I
    applied to x and to the phi-shifted view of x
  - phi-shift is free: x tiles carry 2 halo columns kept up to date with two
    tiny copies per step, so the shifted operand is just a strided view
  - per-step DVE: tmp = psum * g_e ; x' = c1*x + tmp   (g_e = 0.7*decay*af
    host-prepacked); step accumulation acc += w_s * x' runs on GPSIMD

Batch is sharded 2-per-core across 8 cores; no collectives needed.
"""

import numpy as np

NT, NP, S = 128, 64, 10
N = NT * NP
B = 16
NCORES = 8
BPC = B // NCORES  # batches per core
NH = NP + 2        # phi width incl. wrap halos: [wrap_pre | 0..63 | wrap_post]

_FWD = [(1, 0), (0, 1), (1, 1)]
_REV = [(-1, 0), (0, -1), (-1, -1)]


def _diag_vals(adj, shifts):
    idx = np.arange(N)
    ti, pi = idx // NP, idx % NP
    return [adj[idx, ((ti + dt) % NT) * NP + (pi + dp) % NP] for dt, dp in shifts]


def _softmax(x):
    e = np.exp(x - x.max())
    return (e / e.sum()).astype(np.float32)


def _structure_ok(adj, vals):
    for v in vals:
        if np.ptp(v) > 1e-6 * max(1.0, abs(float(v.mean()))):
            return False
    total = adj.sum(dtype=np.float64)
    diag = sum(v.sum(dtype=np.float64) for v in vals)
    return abs(total - diag) < 1e-3


def _reference_fallback(entry, fwd_adj, rev_adj, fwd_sw, fwd_decay, rev_sw,
                        rev_decay, iw, angles):
    # generic dense path (host); only used if the adjacency is not the
    # expected toroidal shift structure.
    def prop(adj, decay, sw):
        d = float(np.clip(decay, 0.5, 0.99))
        af = 0.5 + 0.5 * np.cos(np.abs(angles).mean(axis=1))
        x = entry.astype(np.float32)
        w = _softmax(np.asarray(sw, np.float32))
        acc = np.zeros_like(x)
        for s in range(S):
            p = (x @ adj) * af[None, :]
            x = ((0.3 * x + 0.7 * p) * d).astype(np.float32)
            acc += w[s] * x
        return acc
    f = prop(fwd_adj, fwd_decay, fwd_sw)
    r = prop(rev_adj, rev_decay, rev_sw)
    inter = f * r
    sig = 1.0 / (1.0 + np.exp(-float(iw)))
    return (f + r + np.float32(sig) * inter).astype(np.float32), inter.astype(np.float32)


def _build_program(c1, w, sig_w, use_fp32r=True):
    """SPMD Bass program (identical on all cores).

    consts layout (free dim):
      [P_f | M_f | P_r | M_r  (128 each) | g_e 2*BPC*64 | x0 2*BPC*NH]
    """
    import concourse.bacc as bacc
    import concourse.mybir as mybir
    from concourse.tile import TileContext

    fp32 = mybir.dt.float32
    mm_dt = mybir.dt.float32r if use_fp32r else fp32
    OP = mybir.AluOpType

    nc = bacc.Bacc(None, target_bir_lowering=False)

    GE = 2 * BPC * NP
    XW = 2 * BPC * NH
    CW = 4 * NT + GE + XW
    consts_d = nc.dram_tensor("consts", [NT, CW], fp32, kind="ExternalInput")
    out_d = nc.dram_tensor("out_all", [2, BPC, N], fp32, kind="ExternalOutput")
    MATS = 4 * NT

    (c1f, c1r), (wf, wr) = c1, w
    same_c1 = abs(c1f - c1r) < 1e-12
    same_w = all(abs(a - b) < 1e-12 for a, b in zip(wf, wr))

    g_off = 4 * NT
    x0_off = g_off + GE

    with TileContext(nc) as tc:
        with (
            tc.tile_pool(name="const", bufs=1) as cpool,
            tc.tile_pool(name="state", bufs=2) as spool,
            tc.tile_pool(name="work", bufs=2) as wpool,
            tc.tile_pool(name="accp", bufs=2) as apool,
            tc.tile_pool(name="psum", bufs=2, space="PSUM") as ppool,
        ):
            # three parallel DMAs so the first matmul isn't gated on the
            # whole constant block arriving
            consts = cpool.tile([NT, CW], fp32, tag="consts")
            nc.sync.dma_start(consts[:, 0:MATS], consts_d[:, 0:MATS])
            nc.sync.dma_start(consts[:, x0_off:x0_off + XW],
                              consts_d[:, x0_off:x0_off + XW])
            nc.sync.dma_start(consts[:, g_off:g_off + GE],
                              consts_d[:, g_off:g_off + GE])
            pm = {k: consts[:, i * NT:(i + 1) * NT].bitcast(mm_dt)
                  for i, k in enumerate(("Pf", "Mf", "Pr", "Mr"))}
            g_e = consts[:, g_off:g_off + GE].rearrange(
                "t (d b p) -> t d b p", d=2, b=BPC)

            # initial state: host-packed with halos, read in place
            x = consts[:, x0_off:x0_off + XW].rearrange(
                "t (d b p) -> t d b p", d=2, b=BPC)

            # fully independent fwd / rev chains so one direction's DVE work
            # overlaps the other direction's matmuls
            DIRS = (
                dict(d=0, P="Pf", M="Mf", c1=c1f, w=wf, xtag="xf"),
                dict(d=1, P="Pr", M="Mr", c1=c1r, w=wr, xtag="xr"),
            )
            xs = [x[:, 0], x[:, 1]]           # [128, BPC, NH] views
            accs = [None, None]
            for s in range(S):
                for dd in DIRS:
                    d = dd["d"]
                    xd = xs[d]
                    # prop = P @ x + M @ xphi  (xphi: fwd cols 0..63, rev 2..65)
                    ps = ppool.tile([NT, BPC, NP], fp32, tag=f"ps{d}")
                    xm = xd.bitcast(mm_dt)
                    lo = 0 if d == 0 else 2
                    nc.tensor.matmul(ps[:], pm[dd["P"]], xm[:, :, 1:NP + 1],
                                     start=True, stop=False)
                    nc.tensor.matmul(ps[:], pm[dd["M"]], xm[:, :, lo:lo + NP],
                                     start=False, stop=True)

                    # tmp = prop * g_e ; x'_center = c1*x + tmp
                    tmp = wpool.tile([NT, BPC, NP], fp32, tag=f"tmp{d}")
                    nc.vector.tensor_mul(tmp[:], ps[:], g_e[:, d])
                    xn = spool.tile([NT, BPC, NH], fp32, tag=dd["xtag"])
                    xc = xn[:, :, 1:NP + 1]
                    nc.vector.scalar_tensor_tensor(
                        xc, xd[:, :, 1:NP + 1], dd["c1"], tmp[:],
                        op0=OP.mult, op1=OP.add)
                    # refresh wrap halos (tiny)
                    nc.gpsimd.tensor_copy(xn[:, :, 0:1], xn[:, :, NP:NP + 1])
                    nc.gpsimd.tensor_copy(xn[:, :, NP + 1:NP + 2], xn[:, :, 1:2])
                    xs[d] = xn[:]

                    # acc += w_s * x'
                    an = apool.tile([NT, BPC, NP], fp32, tag=f"acc{d}")
                    if accs[d] is None:
                        nc.vector.tensor_scalar_mul(an[:], xc, dd["w"][0])
                    else:
                        nc.vector.scalar_tensor_tensor(
                            an[:], xc, dd["w"][s], accs[d][:],
                            op0=OP.mult, op1=OP.add)
                    accs[d] = an

            # combine: inter = f*r ; comb = f + r + sig*inter ; one output DMA
            out2 = wpool.tile([NT, 2, BPC, NP], fp32, tag="out2")
            inter = out2[:, 1]
            nc.vector.tensor_mul(inter, accs[0][:], accs[1][:])
            fr = wpool.tile([NT, BPC, NP], fp32, tag="fr")
            nc.vector.tensor_add(fr[:], accs[0][:], accs[1][:])
            nc.vector.scalar_tensor_tensor(
                out2[:, 0], inter, sig_w, fr[:], op0=OP.mult, op1=OP.add)
            nc.sync.dma_start(
                out_d[:].rearrange("o b (t p) -> t o b p", t=NT), out2[:])

    nc.finalize()
    return nc


def _host_prep(inputs):
    entry = np.ascontiguousarray(np.asarray(inputs["entry_probs"], np.float32))
    fwd_adj = np.asarray(inputs["forward_adj"], np.float32)
    rev_adj = np.asarray(inputs["reverse_adj"], np.float32)
    angles = np.asarray(inputs["bounce_angles"], np.float32)

    vf = _diag_vals(fwd_adj, _FWD)
    vr = _diag_vals(rev_adj, _REV)
    ok = _structure_ok(fwd_adj, vf) and _structure_ok(rev_adj, vr)

    df = float(np.clip(float(np.asarray(inputs["forward_decay"])), 0.5, 0.99))
    dr = float(np.clip(float(np.asarray(inputs["reverse_decay"])), 0.5, 0.99))
    wf = _softmax(np.asarray(inputs["forward_step_weights"], np.float32))
    wr = _softmax(np.asarray(inputs["reverse_step_weights"], np.float32))
    sig = float(1.0 / (1.0 + np.exp(-float(np.asarray(inputs["interaction_weight"])))))

    vbf = [float(v.mean()) for v in vf]   # [v10, v01, v11]
    vbr = [float(v.mean()) for v in vr]

    k = np.arange(NT)
    p_fwd = np.zeros((NT, NT), np.float32)   # out[t] = in[t-1]
    p_fwd[(k - 1) % NT, k] = 1.0
    p_rev = np.zeros((NT, NT), np.float32)   # out[t] = in[t+1]
    p_rev[(k + 1) % NT, k] = 1.0
    eye = np.eye(NT, dtype=np.float32)

    mats = [vbf[0] * p_fwd, vbf[2] * p_fwd + vbf[1] * eye,
            vbr[0] * p_rev, vbr[2] * p_rev + vbr[1] * eye]

    # per-cell gain g = 0.7 * decay * (0.5 + 0.5*cos(mean|angles|)),
    # expanded to [128, dir, b, 64]
    af = (0.5 + 0.5 * np.cos(np.abs(angles).mean(axis=1))).astype(np.float32)
    af2 = af.reshape(NT, NP)
    g_e = np.empty((NT, 2, BPC, NP), np.float32)
    g_e[:, 0] = (0.7 * df) * af2[:, None, :]
    g_e[:, 1] = (0.7 * dr) * af2[:, None, :]

    consts_common = np.concatenate(
        [m for m in mats] + [g_e.reshape(NT, -1)], axis=1)

    consts_list = []
    for c in range(NCORES):
        e = entry[c * BPC:(c + 1) * BPC].reshape(BPC, NT, NP).transpose(1, 0, 2)
        x0 = np.empty((NT, 2, BPC, NH), np.float32)
        x0[:, :, :, 1:NP + 1] = e[:, None]
        x0[:, :, :, 0] = e[:, None, :, NP - 1]
        x0[:, :, :, NP + 1] = e[:, None, :, 0]
        consts_list.append(np.ascontiguousarray(
            np.concatenate([consts_common, x0.reshape(NT, -1)], axis=1)))

    meta = dict(
        ok=ok,
        c1=(0.3 * df, 0.3 * dr),
        w=(list(map(float, wf)), list(map(float, wr))),
        sig=sig, consts_list=consts_list,
    )
    return meta


_PROGRAM_CACHE = {}
LAST_RESULT = None
USE_FP32R = False


def kernel(**inputs):
    meta = _host_prep(inputs)
    if not meta["ok"]:
        return _reference_fallback(
            np.asarray(inputs["entry_probs"], np.float32),
            np.asarray(inputs["forward_adj"], np.float32),
            np.asarray(inputs["reverse_adj"], np.float32),
            inputs["forward_step_weights"], inputs["forward_decay"],
            inputs["reverse_step_weights"], inputs["reverse_decay"],
            inputs["interaction_weight"], np.asarray(inputs["bounce_angles"], np.float32))

    from concourse import bass_utils

    key = (tuple(meta["c1"]), tuple(meta["w"][0]), tuple(meta["w"][1]),
           meta["sig"], USE_FP32R)
    if key not in _PROGRAM_CACHE:
        _PROGRAM_CACHE[key] = _build_program(
            meta["c1"], meta["w"], meta["sig"], use_fp32r=USE_FP32R)
    nc = _PROGRAM_CACHE[key]

    in_maps = [{"consts": meta["consts_list"][c]} for c in range(NCORES)]
    res = bass_utils.run_bass_kernel_spmd(nc, in_maps, core_ids=list(range(NCORES)))
    global LAST_RESULT
    LAST_RESULT = res

    combined = np.concatenate([r["out_all"][0] for r in res.results], axis=0)
    interaction = np.concatenate([r["out_all"][1] for r in res.results], axis=0)
    return combined, interaction
